# revision 1
# baseline (speedup 1.0000x reference)
"""DCNv2 deformable PS-RoI pooling on 8 Trainium2 NeuronCores.

Strategy (RoI-data-parallel, 32 rois per core):
  * Host replicates the reference coordinate math exactly in float32 and folds
    bilinear weights, validity masking and the 1/count normalization into a
    per-roi sparse matrix A (bbox_pixels x 49). Each roi touches only a small
    bbox of the 64x64 feature map, so A has ~128-384 rows (padded to 128k).
  * Feature map is transposed to channel-last (B*H*W, C) on host so each pixel
    is a contiguous 1KB channel vector in HBM.
  * Device (SPMD, identical program on 8 cores, per-core data in DRAM inputs):
      - one indirect-DMA gather per column group: patch[p, t, :] =
        Fcl[idx[p, t], :]  -> pixel-on-partition layout
      - per roi: out(c,j) accumulated in PSUM over 128-pixel chunks via
        matmul(lhsT=patch_chunk(128px, 128c), rhs=A_chunk(128px, 49j))
      - PSUM -> SBUF staging -> one contiguous DMA to HBM (c-major scratch
        layout); host undoes the layout permutation while assembling.
"""
import numpy as np

f32 = np.float32
f64 = np.float64

B, C, H, W = 8, 256, 64, 64
N_ROIS, P, S = 256, 7, 4
PART = 7
NJ = P * P  # 49
SCALE = f32(1.0 / 16.0)
TRANS_STD = f32(0.1)
N_CORES = 8
RPC = N_ROIS // N_CORES  # rois per core
N_GROUPS = 6  # gather/compute pipeline groups
GROUP_WEIGHTS = [0.5, 1.4, 1.4, 1.2, 1.0, 0.5, 0.4, 0.3]  # truncated to N_GROUPS
A_SPLIT = False  # upload A per group instead of one shot

_prog_cache = {}


# --------------------------------------------------------------------------
# host math: exact f32 replication of the reference coordinate computation
# --------------------------------------------------------------------------
def _roi_sampling_data(rois, offset):
    rois = np.asarray(rois, dtype=f32)
    offset = np.asarray(offset, dtype=f32)
    batch = rois[:, 0].astype(np.int32)

    roi_sw = np.round(rois[:, 1]) * SCALE - f32(0.5)
    roi_sh = np.round(rois[:, 2]) * SCALE - f32(0.5)
    roi_ew = (np.round(rois[:, 3]) + f32(1.0)) * SCALE - f32(0.5)
    roi_eh = (np.round(rois[:, 4]) + f32(1.0)) * SCALE - f32(0.5)
    roi_w = np.maximum(roi_ew - roi_sw, f32(0.1))
    roi_h = np.maximum(roi_eh - roi_sh, f32(0.1))
    bin_w = roi_w / f32(P)
    bin_h = roi_h / f32(P)
    sub_w = bin_w / f32(S)
    sub_h = bin_h / f32(S)

    ph = np.arange(P, dtype=np.int32)
    pw = np.arange(P, dtype=np.int32)
    part_h = np.clip(
        np.floor(ph.astype(f32) / f32(P) * f32(PART)).astype(np.int32), 0, PART - 1
    )
    part_w = np.clip(
        np.floor(pw.astype(f32) / f32(P) * f32(PART)).astype(np.int32), 0, PART - 1
    )

    tx = offset[:, 0][:, part_h[:, None], part_w[None, :]] * TRANS_STD  # (N,7,7)
    ty = offset[:, 1][:, part_h[:, None], part_w[None, :]] * TRANS_STD

    wstart = (
        pw.astype(f32)[None, None, :] * bin_w[:, None, None]
        + roi_sw[:, None, None]
        + tx * roi_w[:, None, None]
    )
    hstart = (
        ph.astype(f32)[None, :, None] * bin_h[:, None, None]
        + roi_sh[:, None, None]
        + ty * roi_h[:, None, None]
    )

    iw = np.arange(S, dtype=f32)
    ih = np.arange(S, dtype=f32)
    wpos = (
        wstart[:, :, :, None, None]
        + iw[None, None, None, None, :] * sub_w[:, None, None, None, None]
    )
    hpos = (
        hstart[:, :, :, None, None]
        + ih[None, None, None, :, None] * sub_h[:, None, None, None, None]
    )

    valid = (
        (wpos >= f32(-0.5)) & (wpos <= f32(W) - f32(0.5))
        & (hpos >= f32(-0.5)) & (hpos <= f32(H) - f32(0.5))
    )
    wc = np.clip(wpos, f32(0.0), f32(W - 1.0))
    hc = np.clip(hpos, f32(0.0), f32(H - 1.0))

    x0 = np.floor(wc).astype(np.int32)
    x1 = np.ceil(wc).astype(np.int32)
    y0 = np.floor(hc).astype(np.int32)
    y1 = np.ceil(hc).astype(np.int32)
    dx = (wc - np.floor(wc)).astype(f64)
    dy = (hc - np.floor(hc)).astype(f64)

    cnt = valid.sum(axis=(3, 4)).astype(f32)  # (N,7,7)
    coef = np.where(cnt > 0, 1.0 / np.maximum(cnt, f32(1.0)).astype(f64), 0.0)

    w00 = (1.0 - dx) * (1.0 - dy)
    w01 = dx * (1.0 - dy)
    w10 = (1.0 - dx) * dy
    w11 = dx * dy

    return dict(
        batch=batch, valid=valid, x0=x0, x1=x1, y0=y0, y1=y1,
        w00=w00, w01=w01, w10=w10, w11=w11, coef=coef,
    )


def _build_roi_mats(rois, offset):
    """Per roi: (pixel idx int32 (npix,), A f32 (npix, 49)), npix % 128 == 0."""
    d = _roi_sampling_data(rois, offset)
    j_grid = np.arange(NJ, dtype=np.int64).reshape(P, P, 1, 1)
    j_grid = np.broadcast_to(j_grid, (P, P, S, S))
    full = (P, P, S, S)

    out = []
    for n in range(N_ROIS):
        v = d["valid"][n]
        if not v.any():
            out.append((np.zeros(128, np.int32), np.zeros((128, NJ), f32)))
            continue
        jj = j_grid[v]
        xs0 = np.broadcast_to(d["x0"][n], full)[v]
        xs1 = np.broadcast_to(d["x1"][n], full)[v]
        ys0 = np.broadcast_to(d["y0"][n], full)[v]
        ys1 = np.broadcast_to(d["y1"][n], full)[v]
        cf = np.broadcast_to(d["coef"][n][:, :, None, None], full)[v]
        bx0 = int(xs0.min()); bx1 = int(xs1.max())
        by0 = int(ys0.min()); by1 = int(ys1.max())
        bw = bx1 - bx0 + 1
        bh = by1 - by0 + 1
        npix = bh * bw
        npad = (-npix) % 128
        A = np.zeros((npix + npad, NJ), f64)
        for yy, xx, ww in (
            (ys0, xs0, np.broadcast_to(d["w00"][n], full)[v]),
            (ys0, xs1, np.broadcast_to(d["w01"][n], full)[v]),
            (ys1, xs0, np.broadcast_to(d["w10"][n], full)[v]),
            (ys1, xs1, np.broadcast_to(d["w11"][n], full)[v]),
        ):
            lp = (yy - by0).astype(np.int64) * bw + (xx - bx0)
            np.add.at(A, (lp, jj), ww * cf)
        yidx = (by0 + np.arange(bh, dtype=np.int32))[:, None]
        xidx = (bx0 + np.arange(bw, dtype=np.int32))[None, :]
        gidx = (int(d["batch"][n]) * (H * W) + yidx * W + xidx).reshape(-1)
        gidx = np.concatenate([gidx, np.zeros(npad, np.int32)]).astype(np.int32)
        out.append((gidx, A.astype(f32)))
    return out


# --------------------------------------------------------------------------
# device program
# --------------------------------------------------------------------------
def _build_program(nch):
    """nch: tuple of RPC ints = chunks per roi slot. Same program on 8 cores."""
    import concourse.bacc as bacc
    import concourse.bass as bass
    import concourse.mybir as mybir
    from concourse.tile import TileContext

    T = int(sum(nch))
    col0 = np.concatenate([[0], np.cumsum(nch)]).astype(int)  # slot -> first col

    # split slots into N_GROUPS groups; group 0 small so the pipeline
    # starts early, last groups small so the tail drains fast
    weights = GROUP_WEIGHTS[:N_GROUPS]
    cum = np.cumsum(weights) / sum(weights)
    bounds = [0]
    for g in range(N_GROUPS - 1):
        target = T * cum[g]
        s = int(np.searchsorted(col0, target))
        s = min(max(s, bounds[-1] + 1), RPC - (N_GROUPS - 1 - g))
        bounds.append(s)
    bounds.append(RPC)

    nc = bacc.Bacc("TRN2", num_devices=N_CORES)
    dt = mybir.dt
    fcl = nc.dram_tensor("fcl", [B * H * W, C], dt.float16, kind="ExternalInput")
    amat = nc.dram_tensor("amat", [128, T, NJ], dt.float16, kind="ExternalInput")
    # dma_gather index layout: logical idx i lives at [i % 16, i // 16],
    # replicated across the 8 groups of 16 partitions.
    pidx = nc.dram_tensor("pidx", [128, T * 8], dt.int16, kind="ExternalInput")
    outd = nc.dram_tensor("out", [128, RPC, 2, NJ], dt.float16, kind="ExternalOutput")

    with TileContext(nc) as tc:
        with (
            tc.tile_pool(name="main", bufs=1) as mp,
            tc.tile_pool(name="psum", bufs=2, space="PSUM") as pp,
        ):
            idx_t = mp.tile([128, T * 8], dt.int16, tag="idx")
            nc.sync.dma_start(out=idx_t[:], in_=pidx[:])
            if not A_SPLIT:
                a_full = mp.tile([128, T, NJ], dt.float16, tag="amat")
                nc.sync.dma_start(out=a_full[:], in_=amat[:])

            for g in range(N_GROUPS):
                s0, s1 = bounds[g], bounds[g + 1]
                c0, c1 = int(col0[s0]), int(col0[s1])
                ncols = c1 - c0
                if A_SPLIT:
                    a_g = mp.tile([128, ncols, NJ], dt.float16, tag=f"amat{g}")
                    nc.scalar.dma_start(out=a_g[:], in_=amat[:, c0:c1, :])
                p_t = mp.tile([128, ncols, C], dt.float16, tag=f"patch{g}")
                nc.gpsimd.dma_gather(
                    out_ap=p_t[:],
                    in_ap=fcl[:],
                    idxs_ap=idx_t[:, c0 * 8:c1 * 8],
                    num_idxs=ncols * 128,
                    num_idxs_reg=ncols * 128,
                    elem_size=C,
                    single_packet=False,
                )
                ob = mp.tile([128, s1 - s0, 2, NJ], dt.float16, tag=f"outbuf{g}")
                # pack 5 rois (10 roi-halves x 49) per PSUM bank; one DVE
                # copy per bank instead of one per roi-half
                for b0 in range(s0, s1, 5):
                    b1 = min(b0 + 5, s1)
                    nsl = (b1 - b0) * 2
                    pb = pp.tile([128, nsl * NJ], dt.float32, tag="pbank")
                    for r in range(b0, b1):
                        for h in range(2):
                            o = ((r - b0) * 2 + h) * NJ
                            for t in range(nch[r]):
                                c = int(col0[r]) + t
                                rhs = (
                                    a_g[:, c - c0, :] if A_SPLIT
                                    else a_full[:, c, :]
                                )
                                nc.tensor.matmul(
                                    out=pb[:, o:o + NJ],
                                    lhsT=p_t[:, c - c0, h * 128:(h + 1) * 128],
                                    rhs=rhs,
                                    start=(t == 0),
                                    stop=(t == nch[r] - 1),
                                )
                    nc.vector.tensor_copy(
                        out=ob[:, b0 - s0:b1 - s0, :, :], in_=pb[:, :nsl * NJ]
                    )
                # one output DMA per group; the last group drains per-bank
                # via the loop above having filled ob fully
                nc.sync.dma_start(out=outd[:, s0:s1, :, :], in_=ob[:])
    nc.compile()
    return nc


# --------------------------------------------------------------------------
# entry point
# --------------------------------------------------------------------------
def _partition_rois(mats):
    """Snake-deal rois to cores by descending chunk count so every slot r
    holds 8 near-equal-size rois -> per-slot max (nch) is tight."""
    chunks_per = np.array([len(g) // 128 for g, _ in mats])
    order = np.argsort(-chunks_per, kind="stable")
    slots = [[None] * RPC for _ in range(N_CORES)]  # slots[k][r] = roi index
    for i, roi in enumerate(order):
        rnd, pos = divmod(i, N_CORES)
        core = pos if rnd % 2 == 0 else N_CORES - 1 - pos
        slots[core][rnd] = int(roi)
    slots = [np.array(s) for s in slots]
    nch = tuple(
        int(max(chunks_per[slots[k][r]] for k in range(N_CORES))) for r in range(RPC)
    )
    return slots, nch


def kernel(input, rois, offset):
    from concourse.bass_utils import run_bass_kernel_spmd

    input = np.asarray(input, dtype=f32)
    mats = _build_roi_mats(rois, offset)

    fcl = np.ascontiguousarray(
        input.transpose(0, 2, 3, 1).astype(np.float16)
    ).reshape(B * H * W, C)

    slots, nch = _partition_rois(mats)
    T = int(sum(nch))
    col0 = np.concatenate([[0], np.cumsum(nch)]).astype(int)

    key = nch
    if key not in _prog_cache:
        _prog_cache[key] = _build_program(nch)
    nc = _prog_cache[key]

    in_maps = []
    for k in range(N_CORES):
        logical = np.zeros(T * 128, np.int32)
        a_arr = np.zeros((128, T, NJ), np.float16)
        for r in range(RPC):
            gidx, A = mats[slots[k][r]]
            tchunks = len(gidx) // 128
            for t in range(tchunks):
                col = int(col0[r]) + t
                logical[col * 128:(col + 1) * 128] = gidx[t * 128:(t + 1) * 128]
                a_arr[:, col, :] = A[t * 128:(t + 1) * 128, :]
        # wrap-16 + replicate to 128 partitions (see _build_program)
        idx16 = np.tile(logical.astype(np.int16).reshape(-1, 16).T, (8, 1))
        in_maps.append({"fcl": fcl, "amat": a_arr, "pidx": idx16})

    res = run_bass_kernel_spmd(nc, in_maps, core_ids=list(range(N_CORES)))

    out_full = np.empty((N_ROIS, C, P, P), f32)
    for k in range(N_CORES):
        arr = res.results[k]["out"].astype(f32)  # (128, RPC, 2, 49)
        t = arr.transpose(1, 2, 0, 3).reshape(RPC, C, P, P)
        out_full[slots[k]] = t
    return out_full



# revision 5
# speedup vs baseline: 1.0067x; 1.0067x over previous
"""DCNv2 deformable PS-RoI pooling on 8 Trainium2 NeuronCores (v2).

Strategy (RoI-data-parallel, 32 rois per core):
  * Host replicates the reference coordinate math exactly in float32.
    Bilinear weights / validity / 1-over-count factor per-bin separably:
    A[(y,x), j] = Wy[y, j] * Wx[x, j], so only pixels with
    (Wy row nonzero) x (Wx col nonzero) are needed -- the touched set is
    exactly a cartesian product ys x xs (~55% of the padded bbox).
  * Touched pixels of the 32 rois are bin-packed densely into 128-row
    chunks (per psum-group of rois), removing per-roi 128-padding.
  * Device (SPMD): indirect-DMA gather of pixel channel vectors
    (512B each, full DMA efficiency), matmul patch^T @ A per chunk into
    per-group PSUM banks, PSUM->SBUF copies split across DVE/Act,
    per-group output DMAs. All DMACopies dispatch from the Pool queue
    (cheap 25ns dispatch).
"""
import numpy as np

f32 = np.float32
f64 = np.float64

B, C, H, W = 8, 256, 64, 64
N_ROIS, P, S = 256, 7, 4
NJ = P * P  # 49
SCALE = f32(1.0 / 16.0)
TRANS_STD = f32(0.1)
N_CORES = 8
RPC = N_ROIS // N_CORES  # 32 rois per core
GROUP_SIZES = (10, 10, 10, 2)  # psum groups (<=10 per half-bank: 10*49<=512)
# ranks (desc size) -> group assignment: moderate first, big middle, tiny last
GROUP_RANKS = (tuple(range(20, 30)), tuple(range(0, 10)),
               tuple(range(10, 20)), (30, 31))
N_GCALLS = 3  # gather calls: group0 | group1 | group2+group3

_prog_cache = {}


# --------------------------------------------------------------------------
# host math: exact f32 replication, separable per-bin weights
# --------------------------------------------------------------------------
def _sep_weights(rois, offset):
    """Per roi: (batch, ys, xs, Wy (ny,49) f64, Wx (nx,49) f64)."""
    rois = np.asarray(rois, dtype=f32)
    offset = np.asarray(offset, dtype=f32)
    N = rois.shape[0]
    batch = rois[:, 0].astype(np.int32)

    roi_sw = np.round(rois[:, 1]) * SCALE - f32(0.5)
    roi_sh = np.round(rois[:, 2]) * SCALE - f32(0.5)
    roi_ew = (np.round(rois[:, 3]) + f32(1.0)) * SCALE - f32(0.5)
    roi_eh = (np.round(rois[:, 4]) + f32(1.0)) * SCALE - f32(0.5)
    roi_w = np.maximum(roi_ew - roi_sw, f32(0.1))
    roi_h = np.maximum(roi_eh - roi_sh, f32(0.1))
    bin_w = roi_w / f32(P)
    bin_h = roi_h / f32(P)
    sub_w = bin_w / f32(S)
    sub_h = bin_h / f32(S)

    ph = np.arange(P, dtype=np.int32)
    pw = np.arange(P, dtype=np.int32)
    part_h = np.clip(
        np.floor(ph.astype(f32) / f32(P) * f32(P)).astype(np.int32), 0, P - 1
    )
    part_w = np.clip(
        np.floor(pw.astype(f32) / f32(P) * f32(P)).astype(np.int32), 0, P - 1
    )
    tx = offset[:, 0][:, part_h[:, None], part_w[None, :]] * TRANS_STD  # (N,7,7)
    ty = offset[:, 1][:, part_h[:, None], part_w[None, :]] * TRANS_STD

    wstart = (
        pw.astype(f32)[None, None, :] * bin_w[:, None, None]
        + roi_sw[:, None, None]
        + tx * roi_w[:, None, None]
    )  # (N,7,7)
    hstart = (
        ph.astype(f32)[None, :, None] * bin_h[:, None, None]
        + roi_sh[:, None, None]
        + ty * roi_h[:, None, None]
    )

    samp = np.arange(S, dtype=f32)
    wpos = wstart[..., None] + samp * sub_w[:, None, None, None]  # (N,7,7,4)
    hpos = hstart[..., None] + samp * sub_h[:, None, None, None]

    vw = (wpos >= f32(-0.5)) & (wpos <= f32(W) - f32(0.5))
    vh = (hpos >= f32(-0.5)) & (hpos <= f32(H) - f32(0.5))
    wc = np.clip(wpos, f32(0.0), f32(W - 1.0))
    hc = np.clip(hpos, f32(0.0), f32(H - 1.0))

    x0 = np.floor(wc).astype(np.int64)
    x1 = np.ceil(wc).astype(np.int64)
    y0 = np.floor(hc).astype(np.int64)
    y1 = np.ceil(hc).astype(np.int64)
    dx = (wc - np.floor(wc)).astype(f64)
    dy = (hc - np.floor(hc)).astype(f64)

    cnt_h = vh.sum(axis=3)  # (N,7,7)
    cnt_w = vw.sum(axis=3)
    ch = 1.0 / np.maximum(cnt_h, 1).astype(f64)
    cw = 1.0 / np.maximum(cnt_w, 1).astype(f64)

    jidx = (ph[:, None] * P + pw[None, :]).astype(np.int64)  # (7,7)
    jb = np.broadcast_to(jidx[None, :, :, None], (N, P, P, S))
    nb = np.broadcast_to(np.arange(N, dtype=np.int64)[:, None, None, None],
                         (N, P, P, S))

    Wy_full = np.zeros((N, H, NJ), f64)
    why0 = (1.0 - dy) * vh * ch[..., None]
    why1 = dy * vh * ch[..., None]
    np.add.at(Wy_full, (nb, y0, jb), why0)
    np.add.at(Wy_full, (nb, y1, jb), why1)

    Wx_full = np.zeros((N, W, NJ), f64)
    wwx0 = (1.0 - dx) * vw * cw[..., None]
    wwx1 = dx * vw * cw[..., None]
    np.add.at(Wx_full, (nb, x0, jb), wwx0)
    np.add.at(Wx_full, (nb, x1, jb), wwx1)

    out = []
    for n in range(N):
        ys = np.nonzero(np.abs(Wy_full[n]).sum(axis=1) > 0)[0]
        xs = np.nonzero(np.abs(Wx_full[n]).sum(axis=1) > 0)[0]
        out.append((int(batch[n]), ys.astype(np.int32), xs.astype(np.int32),
                    Wy_full[n][ys], Wx_full[n][xs]))
    return out


# --------------------------------------------------------------------------
# packing plan (shared structure across the 8 cores)
# --------------------------------------------------------------------------
def _make_plan(npads):
    """npads: tuple of 32 ints (desc), rank r -> padded pixel count.

    Returns dict with:
      rank2slot: rank -> output slot index o (0..31), slot o in group
                 g with position pos (o = base[g] + pos)
      chunks: list of (gcall, [segments]) in stream order; segment =
              (rank, row0, row1, pix0, start, stop, pair)
      gcall_nch: chunks per gather call; pairs: total pair count
      mms: list per chunk of merged matmuls (pair0, npair, g, pos0,
           start, stop)
    """
    base = np.concatenate([[0], np.cumsum(GROUP_SIZES)])
    rank2slot = {}
    chunks = []
    seg_of_slot = {}
    pair = 0
    gcall_nch = []
    gcall_of_group = (0, 1, 2, 2)
    cur_gcall_count = {0: 0, 1: 1, 2: 2}
    for g, ranks in enumerate(GROUP_RANKS):
        # position within group: pack order (desc npad)
        ranks_sorted = sorted(ranks, key=lambda r: -npads[r])
        for pos, r in enumerate(ranks_sorted):
            rank2slot[r] = int(base[g] + pos)
        gchunks = []
        # dedicated full chunks
        rems = []
        for r in ranks_sorted:
            npad = npads[r]
            nfull, rem = divmod(npad, 128)
            for i in range(nfull):
                st = i == 0
                sp = (i == nfull - 1) and rem == 0
                gchunks.append([(r, 0, 128, i * 128, st, sp)])
            if rem:
                rems.append((r, rem, nfull))
            elif nfull == 0:
                # empty roi: give it a zero-width segment in no chunk; its
                # psum cols never get a matmul -> bank garbage. Avoid by
                # forcing a 1-row segment later via rems with rem>=1.
                rems.append((r, 1, 0))
        # first-fit-decreasing bins for remainders
        rems.sort(key=lambda t: -t[1])
        bins = []  # (used, [segments])
        for r, rem, nfull in rems:
            placed = False
            for b in bins:
                if b[0] + rem <= 128:
                    b[1].append((r, b[0], b[0] + rem, nfull * 128,
                                 nfull == 0, True))
                    b[0] += rem
                    placed = True
                    break
            if not placed:
                bins.append([rem, [(r, 0, rem, nfull * 128, nfull == 0, True)]])
        gchunks.extend(b[1] for b in bins)
        gc = gcall_of_group[g] if g < 3 else 2
        for segs in gchunks:
            chunks.append((gc, segs))
    # order chunks by gcall (groups are already in gcall order)
    # assign pair ids in final chunk order
    final_chunks = []
    for gc, segs in chunks:
        segs2 = []
        for (r, r0, r1, px, st, sp) in segs:
            segs2.append((r, r0, r1, px, st, sp, pair))
            pair += 1
        final_chunks.append((gc, segs2))
    gcall_nch = [sum(1 for gc, _ in final_chunks if gc == i)
                 for i in range(N_GCALLS)]

    # PSUM zero-region rule: start=True marks the whole 2KB bank pending --
    # a bank's accumulation chains must be emitted strictly sequentially.
    # Build per-group, per-slot chains: chains[g] = list (pos order) of
    # [(chunk_idx, pair), ...] in accumulation order.
    slot_group = {}
    slot_pos = {}
    for g, ranks in enumerate(GROUP_RANKS):
        for r in ranks:
            o = rank2slot[r]
            slot_group[r] = g
            slot_pos[r] = int(o - base[g])
    chain_of_rank = {r: [] for r in range(len(npads))}
    for ci, (gc, segs) in enumerate(final_chunks):
        for (r, r0, r1, px, st, sp, pr) in segs:
            chain_of_rank[r].append((px, ci, pr))
    chains = []
    for g, ranks in enumerate(GROUP_RANKS):
        glist = [None] * len(ranks)
        for r in ranks:
            glist[slot_pos[r]] = [(ci, pr)
                                  for (px, ci, pr) in sorted(chain_of_rank[r])]
        chains.append(glist)

    return dict(rank2slot=rank2slot, chunks=final_chunks,
                gcall_nch=gcall_nch, pairs=pair, chains=chains,
                base=[int(b) for b in base])


# --------------------------------------------------------------------------
# device program
# --------------------------------------------------------------------------
def _build_program(npads):
    import concourse.bacc as bacc
    import concourse.mybir as mybir
    from concourse.tile import TileContext

    plan = _make_plan(npads)
    T = len(plan["chunks"])
    PAIRS = plan["pairs"]
    gcall_nch = plan["gcall_nch"]
    base = plan["base"]

    nc = bacc.Bacc("TRN2", num_devices=N_CORES)
    dt = mybir.dt
    fcl = nc.dram_tensor("fcl", [B * H * W, C], dt.float16, kind="ExternalInput")
    amat = nc.dram_tensor("amat", [128, PAIRS, NJ], dt.float16,
                          kind="ExternalInput")
    pidx = nc.dram_tensor("pidx", [128, T * 8], dt.int16, kind="ExternalInput")
    outd = nc.dram_tensor("out", [128, 2, RPC, NJ], dt.float16,
                          kind="ExternalOutput")

    gc_chunk0 = [0, gcall_nch[0], gcall_nch[0] + gcall_nch[1], T]

    with TileContext(nc) as tc:
        with (
            tc.tile_pool(name="main", bufs=1) as mp,
            tc.tile_pool(name="psum", bufs=1, space="PSUM") as pp,
        ):
            idx_t = mp.tile([128, T * 8], dt.int16, tag="idx")
            a_t = mp.tile([128, PAIRS, NJ], dt.float16, tag="amat")
            ob = mp.tile([128, 2, RPC, NJ], dt.float16, tag="outbuf")
            nc.gpsimd.dma_start(out=idx_t[:], in_=pidx[:])
            nc.gpsimd.dma_start(out=a_t[:], in_=amat[:])

            patch = []
            for gc in range(N_GCALLS):
                nch = gcall_nch[gc]
                p_t = mp.tile([128, nch, C], dt.float16, tag=f"patch{gc}")
                c0 = gc_chunk0[gc]
                nc.gpsimd.dma_gather(
                    out_ap=p_t[:],
                    in_ap=fcl[:],
                    idxs_ap=idx_t[:, c0 * 8:(c0 + nch) * 8],
                    num_idxs=nch * 128,
                    num_idxs_reg=nch * 128,
                    elem_size=C,
                    single_packet=False,
                )
                patch.append(p_t)

            pb = {}
            for g, gs in enumerate(GROUP_SIZES):
                for h in range(2):
                    pb[(g, h)] = pp.tile([128, gs, NJ], dt.float32,
                                         tag=f"pb{g}_{h}",
                                         name=f"pb{g}_{h}")

            chunk_gc = [gc for gc, _ in plan["chunks"]]
            for g in range(len(GROUP_SIZES)):
                for pos, chain in enumerate(plan["chains"][g]):
                    for h in range(2):
                        for i, (ci, pr) in enumerate(chain):
                            gc = chunk_gc[ci]
                            lc = ci - gc_chunk0[gc]
                            nc.tensor.matmul(
                                out=pb[(g, h)][:, pos, :],
                                lhsT=patch[gc][:, lc, h * 128:(h + 1) * 128],
                                rhs=a_t[:, pr, :],
                                start=(i == 0),
                                stop=(i == len(chain) - 1),
                            )

            for g, gs in enumerate(GROUP_SIZES):
                o0 = base[g]
                nc.vector.tensor_copy(out=ob[:, 0, o0:o0 + gs, :],
                                      in_=pb[(g, 0)][:])
                nc.scalar.activation(ob[:, 1, o0:o0 + gs, :], pb[(g, 1)][:],
                                     mybir.ActivationFunctionType.Copy)
                nc.gpsimd.dma_start(out=outd[:, :, o0:o0 + gs, :],
                                    in_=ob[:, :, o0:o0 + gs, :])
    nc.compile()
    return nc, plan


# --------------------------------------------------------------------------
# entry point
# --------------------------------------------------------------------------
def kernel(input, rois, offset):
    from concourse.bass_utils import run_bass_kernel_spmd

    input = np.asarray(input, dtype=f32)
    wts = _sep_weights(rois, offset)
    npix = np.array([len(t[1]) * len(t[2]) for t in wts])

    # snake-deal rois to cores by desc touched-pixel count
    order = np.argsort(-npix, kind="stable")
    core_rois = [[] for _ in range(N_CORES)]
    for i, roi in enumerate(order):
        rnd, pos = divmod(i, N_CORES)
        core = pos if rnd % 2 == 0 else N_CORES - 1 - pos
        core_rois[core].append(int(roi))
    # per core sort desc -> rank order; npad per rank = max across cores
    for k in range(N_CORES):
        core_rois[k].sort(key=lambda r: -npix[r])
    npads = tuple(
        int(max(max(npix[core_rois[k][r]], 1) for k in range(N_CORES)))
        for r in range(RPC)
    )

    key = npads
    if key not in _prog_cache:
        _prog_cache[key] = _build_program(npads)
    nc, plan = _prog_cache[key]
    T = len(plan["chunks"])
    PAIRS = plan["pairs"]

    fcl = np.ascontiguousarray(
        input.transpose(0, 2, 3, 1).astype(np.float16)
    ).reshape(B * H * W, C)

    in_maps = []
    for k in range(N_CORES):
        logical = np.zeros(T * 128, np.int32)
        a_arr = np.zeros((128, PAIRS, NJ), np.float16)
        for ci, (gc, segs) in enumerate(plan["chunks"]):
            for (r, r0, r1, px0, st, sp, pr) in segs:
                roi = core_rois[k][r]
                b, ys, xs, Wy, Wx = wts[roi]
                ny, nx = len(ys), len(xs)
                np_r = ny * nx
                n = r1 - r0
                pix = px0 + np.arange(n)
                m = pix < np_r
                if not m.any():
                    continue
                pixm = pix[m]
                yy = ys[pixm // nx]
                xx = xs[pixm % nx]
                logical[ci * 128 + r0:ci * 128 + r1][m] = (
                    b * (H * W) + yy.astype(np.int32) * W + xx
                )
                a_arr[np.arange(r0, r1)[m], pr, :] = (
                    Wy[pixm // nx] * Wx[pixm % nx]
                ).astype(np.float16)
        idx16 = np.tile(logical.astype(np.int16).reshape(-1, 16).T, (8, 1))
        in_maps.append({"fcl": fcl, "amat": a_arr, "pidx": idx16})

    res = run_bass_kernel_spmd(nc, in_maps, core_ids=list(range(N_CORES)))

    out_full = np.empty((N_ROIS, C, P, P), f32)
    for k in range(N_CORES):
        arr = res.results[k]["out"].astype(f32)  # (128, 2, RPC, 49)
        t = arr.transpose(2, 1, 0, 3).reshape(RPC, C, P, P)
        for r in range(RPC):
            out_full[core_rois[k][r]] = t[plan["rank2slot"][r]]
    return out_full


# revision 10
# speedup vs baseline: 1.0800x; 1.0728x over previous
"""DCNv2 deformable PS-RoI pooling on 8 Trainium2 NeuronCores (v2).

Strategy (RoI-data-parallel, 32 rois per core):
  * Host replicates the reference coordinate math exactly in float32.
    Bilinear weights / validity / 1-over-count factor per-bin separably:
    A[(y,x), j] = Wy[y, j] * Wx[x, j], so only pixels with
    (Wy row nonzero) x (Wx col nonzero) are needed -- the touched set is
    exactly a cartesian product ys x xs (~55% of the padded bbox).
  * Touched pixels of the 32 rois are bin-packed densely into 128-row
    chunks (per psum-group of rois), removing per-roi 128-padding.
  * Device (SPMD): indirect-DMA gather of pixel channel vectors
    (512B each, full DMA efficiency), matmul patch^T @ A per chunk into
    per-group PSUM banks, PSUM->SBUF copies split across DVE/Act,
    per-group output DMAs. All DMACopies dispatch from the Pool queue
    (cheap 25ns dispatch).
"""
import numpy as np

f32 = np.float32
f64 = np.float64

B, C, H, W = 8, 256, 64, 64
N_ROIS, P, S = 256, 7, 4
NJ = P * P  # 49
SCALE = f32(1.0 / 16.0)
TRANS_STD = f32(0.1)
N_CORES = 8
RPC = N_ROIS // N_CORES  # 32 rois per core
GROUP_SIZES = (10, 10, 10, 2)  # psum groups (<=10 per half-bank: 10*49<=512)
# ranks (desc size) -> groups: snake-deal 0..29 so each group gets a size
# mix (big rems pack with small rois), group3 = 2 smallest
_SNAKE = (0, 1, 2, 2, 1, 0)
GROUP_RANKS = tuple(
    tuple(r for r in range(30) if _SNAKE[r % 6] == g) for g in range(3)
) + ((30, 31),)
N_GCALLS = 4  # gather calls: group0 split in 2 | group1 | group2+group3
GCALL_OF_GROUP = (1, 2, 3, 3)  # group g's chunks end in this gcall
G0_SPLIT = 3  # first gcall = first 3 chunks of group0

_prog_cache = {}


# --------------------------------------------------------------------------
# host math: exact f32 replication, separable per-bin weights
# --------------------------------------------------------------------------
def _sep_weights(rois, offset):
    """Per roi: (batch, ys, xs, Wy (ny,49) f64, Wx (nx,49) f64)."""
    rois = np.asarray(rois, dtype=f32)
    offset = np.asarray(offset, dtype=f32)
    N = rois.shape[0]
    batch = rois[:, 0].astype(np.int32)

    roi_sw = np.round(rois[:, 1]) * SCALE - f32(0.5)
    roi_sh = np.round(rois[:, 2]) * SCALE - f32(0.5)
    roi_ew = (np.round(rois[:, 3]) + f32(1.0)) * SCALE - f32(0.5)
    roi_eh = (np.round(rois[:, 4]) + f32(1.0)) * SCALE - f32(0.5)
    roi_w = np.maximum(roi_ew - roi_sw, f32(0.1))
    roi_h = np.maximum(roi_eh - roi_sh, f32(0.1))
    bin_w = roi_w / f32(P)
    bin_h = roi_h / f32(P)
    sub_w = bin_w / f32(S)
    sub_h = bin_h / f32(S)

    ph = np.arange(P, dtype=np.int32)
    pw = np.arange(P, dtype=np.int32)
    part_h = np.clip(
        np.floor(ph.astype(f32) / f32(P) * f32(P)).astype(np.int32), 0, P - 1
    )
    part_w = np.clip(
        np.floor(pw.astype(f32) / f32(P) * f32(P)).astype(np.int32), 0, P - 1
    )
    tx = offset[:, 0][:, part_h[:, None], part_w[None, :]] * TRANS_STD  # (N,7,7)
    ty = offset[:, 1][:, part_h[:, None], part_w[None, :]] * TRANS_STD

    wstart = (
        pw.astype(f32)[None, None, :] * bin_w[:, None, None]
        + roi_sw[:, None, None]
        + tx * roi_w[:, None, None]
    )  # (N,7,7)
    hstart = (
        ph.astype(f32)[None, :, None] * bin_h[:, None, None]
        + roi_sh[:, None, None]
        + ty * roi_h[:, None, None]
    )

    samp = np.arange(S, dtype=f32)
    wpos = wstart[..., None] + samp * sub_w[:, None, None, None]  # (N,7,7,4)
    hpos = hstart[..., None] + samp * sub_h[:, None, None, None]

    vw = (wpos >= f32(-0.5)) & (wpos <= f32(W) - f32(0.5))
    vh = (hpos >= f32(-0.5)) & (hpos <= f32(H) - f32(0.5))
    wc = np.clip(wpos, f32(0.0), f32(W - 1.0))
    hc = np.clip(hpos, f32(0.0), f32(H - 1.0))

    x0 = np.floor(wc).astype(np.int64)
    x1 = np.ceil(wc).astype(np.int64)
    y0 = np.floor(hc).astype(np.int64)
    y1 = np.ceil(hc).astype(np.int64)
    dx = (wc - np.floor(wc)).astype(f64)
    dy = (hc - np.floor(hc)).astype(f64)

    cnt_h = vh.sum(axis=3)  # (N,7,7)
    cnt_w = vw.sum(axis=3)
    ch = 1.0 / np.maximum(cnt_h, 1).astype(f64)
    cw = 1.0 / np.maximum(cnt_w, 1).astype(f64)

    jidx = (ph[:, None] * P + pw[None, :]).astype(np.int64)  # (7,7)
    jb = np.broadcast_to(jidx[None, :, :, None], (N, P, P, S))
    nb = np.broadcast_to(np.arange(N, dtype=np.int64)[:, None, None, None],
                         (N, P, P, S))

    Wy_full = np.zeros((N, H, NJ), f64)
    why0 = (1.0 - dy) * vh * ch[..., None]
    why1 = dy * vh * ch[..., None]
    np.add.at(Wy_full, (nb, y0, jb), why0)
    np.add.at(Wy_full, (nb, y1, jb), why1)

    Wx_full = np.zeros((N, W, NJ), f64)
    wwx0 = (1.0 - dx) * vw * cw[..., None]
    wwx1 = dx * vw * cw[..., None]
    np.add.at(Wx_full, (nb, x0, jb), wwx0)
    np.add.at(Wx_full, (nb, x1, jb), wwx1)

    out = []
    for n in range(N):
        ys = np.nonzero(np.abs(Wy_full[n]).sum(axis=1) > 0)[0]
        xs = np.nonzero(np.abs(Wx_full[n]).sum(axis=1) > 0)[0]
        out.append((int(batch[n]), ys.astype(np.int32), xs.astype(np.int32),
                    Wy_full[n][ys], Wx_full[n][xs]))
    return out


# --------------------------------------------------------------------------
# packing plan (shared structure across the 8 cores)
# --------------------------------------------------------------------------
def _make_plan(npads):
    """npads: tuple of 32 ints (desc), rank r -> padded pixel count.

    Returns dict with:
      rank2slot: rank -> output slot index o (0..31); slot o in group g
                 at position pos (o = base[g] + pos, pos = order of first
                 appearance in the group's chunk stream)
      chunks: list of [segments] in stream order; segment =
              (rank, row0, row1, pix0, pair)
      gcall_nch: chunks per gather call
      mchains: per group, list (PE-emission order) of
               (pos0, [(chunk, pair0, npair), ...]) accumulation chains
    """
    base = [0]
    for gs in GROUP_SIZES:
        base.append(base[-1] + gs)
    pair = 0
    all_chunks = []   # list of [ (rank, row0, row1, pix0, pair) ]
    group_chunk_rng = []
    rank2slot = {}
    chain_of_rank = {r: [] for r in range(len(npads))}
    for g, ranks in enumerate(GROUP_RANKS):
        c_start = len(all_chunks)
        ranks_sorted = sorted(ranks, key=lambda r: -npads[r])
        gchunks = []
        rems = []
        for r in ranks_sorted:
            nfull, rem = divmod(npads[r], 128)
            for i in range(nfull):
                gchunks.append([(r, 0, 128, i * 128)])
            rems.append((r, max(rem, 1) if nfull == 0 else rem, nfull))
        rems = [t for t in rems if t[1] > 0]
        rems.sort(key=lambda t: -t[1])
        bins = []
        for r, rem, nfull in rems:
            for b in bins:
                if b[0] + rem <= 128:
                    b[1].append((r, b[0], b[0] + rem, nfull * 128))
                    b[0] += rem
                    break
            else:
                bins.append([rem, [(r, 0, rem, nfull * 128)]])
        gchunks.extend(b[1] for b in bins)
        # assign pairs + positions (first appearance order) + chains
        pos_of = {}
        for segs in gchunks:
            ci = len(all_chunks)
            segs2 = []
            for (r, r0, r1, px) in segs:
                if r not in pos_of:
                    pos_of[r] = len(pos_of)
                segs2.append((r, r0, r1, px, pair))
                chain_of_rank[r].append((px, ci, pair))
                pair += 1
            all_chunks.append(segs2)
        for r, pos in pos_of.items():
            rank2slot[r] = base[g] + pos
        group_chunk_rng.append((c_start, len(all_chunks)))

    T = len(all_chunks)
    # gather call boundaries: split group0 at G0_SPLIT, then group1,
    # then groups 2+3
    b0 = min(G0_SPLIT, group_chunk_rng[0][1])
    gc_bounds = [0, b0, group_chunk_rng[0][1], group_chunk_rng[1][1], T]
    gcall_nch = [gc_bounds[i + 1] - gc_bounds[i] for i in range(N_GCALLS)]

    # per-group chains in pos order, then merge runs of single-chunk
    # chains with consecutive (pos, pair) in the same chunk
    mchains = []
    for g, ranks in enumerate(GROUP_RANKS):
        glist = [None] * len(ranks)
        for r in ranks:
            pos = rank2slot[r] - base[g]
            glist[pos] = sorted(chain_of_rank[r])
        merged = []
        pos = 0
        while pos < len(glist):
            ch = glist[pos]
            if len(ch) == 1:
                px, ci, pr = ch[0]
                npair = 1
                while (pos + npair < len(glist)
                       and len(glist[pos + npair]) == 1
                       and glist[pos + npair][0][1] == ci
                       and glist[pos + npair][0][2] == pr + npair):
                    npair += 1
                merged.append((pos, [(ci, pr, npair)]))
                pos += npair
            else:
                merged.append((pos, [(ci, pr, 1) for (px, ci, pr) in ch]))
                pos += 1
        # PE emission order: chains whose data lands earliest first
        merged.sort(key=lambda m: max(it[0] for it in m[1]))
        mchains.append(merged)

    return dict(rank2slot=rank2slot, chunks=all_chunks,
                gcall_nch=gcall_nch, pairs=pair, mchains=mchains,
                base=base)


# --------------------------------------------------------------------------
# device program
# --------------------------------------------------------------------------
def _build_program(npads):
    import concourse.bacc as bacc
    import concourse.mybir as mybir
    from concourse.tile import TileContext

    plan = _make_plan(npads)
    T = len(plan["chunks"])
    PAIRS = plan["pairs"]
    gcall_nch = plan["gcall_nch"]
    base = plan["base"]

    nc = bacc.Bacc("TRN2", num_devices=N_CORES)
    dt = mybir.dt
    fcl = nc.dram_tensor("fcl", [B * H * W, C], dt.float16, kind="ExternalInput")
    amat = nc.dram_tensor("amat", [128, PAIRS, NJ], dt.float16,
                          kind="ExternalInput")
    pidx = nc.dram_tensor("pidx", [128, T * 8], dt.int16, kind="ExternalInput")
    outd = nc.dram_tensor("out", [128, 2, RPC, NJ], dt.float16,
                          kind="ExternalOutput")

    gc_bounds = [0]
    for n in gcall_nch:
        gc_bounds.append(gc_bounds[-1] + n)

    def gc_of(ci):
        for gc in range(N_GCALLS):
            if ci < gc_bounds[gc + 1]:
                return gc
        raise AssertionError(ci)

    with TileContext(nc) as tc:
        with (
            tc.tile_pool(name="main", bufs=1) as mp,
            tc.tile_pool(name="psum", bufs=1, space="PSUM") as pp,
        ):
            idx_t = mp.tile([128, T * 8], dt.int16, tag="idx")
            a_t = mp.tile([128, PAIRS, NJ], dt.float16, tag="amat")
            ob = mp.tile([128, 2, RPC, NJ], dt.float16, tag="outbuf")
            warm = mp.tile([128, 8], dt.float16, tag="warm")
            nc.sync.dma_start(out=idx_t[:], in_=pidx[:])
            nc.sync.dma_start(out=a_t[:], in_=amat[:])
            # preload the Act engine's activation table (1283ns) upfront
            nc.vector.memset(warm[:], 0.0)
            nc.scalar.activation(warm[:], warm[:],
                                 mybir.ActivationFunctionType.Copy)

            patch = []
            for gc in range(N_GCALLS):
                nch = gcall_nch[gc]
                p_t = mp.tile([128, nch, C], dt.float16, tag=f"patch{gc}")
                c0 = gc_bounds[gc]
                nc.gpsimd.dma_gather(
                    out_ap=p_t[:],
                    in_ap=fcl[:],
                    idxs_ap=idx_t[:, c0 * 8:(c0 + nch) * 8],
                    num_idxs=nch * 128,
                    num_idxs_reg=nch * 128,
                    elem_size=C,
                    single_packet=False,
                )
                patch.append(p_t)

            pb = {}
            for g, gs in enumerate(GROUP_SIZES):
                for h in range(2):
                    pb[(g, h)] = pp.tile([128, gs, NJ], dt.float32,
                                         tag=f"pb{g}_{h}",
                                         name=f"pb{g}_{h}")

            for g in range(len(GROUP_SIZES)):
                for (pos0, items) in plan["mchains"][g]:
                    for h in range(2):
                        for i, (ci, pr, npair) in enumerate(items):
                            gc = gc_of(ci)
                            lc = ci - gc_bounds[gc]
                            nc.tensor.matmul(
                                out=pb[(g, h)][:, pos0:pos0 + npair, :],
                                lhsT=patch[gc][:, lc, h * 128:(h + 1) * 128],
                                rhs=a_t[:, pr:pr + npair, :],
                                start=(i == 0),
                                stop=(i == len(items) - 1),
                            )

            for g, gs in enumerate(GROUP_SIZES):
                o0 = base[g]
                nc.vector.tensor_copy(out=ob[:, 0, o0:o0 + gs, :],
                                      in_=pb[(g, 0)][:])
                nc.scalar.activation(ob[:, 1, o0:o0 + gs, :], pb[(g, 1)][:],
                                     mybir.ActivationFunctionType.Copy)
                nc.sync.dma_start(out=outd[:, :, o0:o0 + gs, :],
                                  in_=ob[:, :, o0:o0 + gs, :])
    nc.compile()
    return nc, plan


# --------------------------------------------------------------------------
# entry point
# --------------------------------------------------------------------------
def kernel(input, rois, offset):
    from concourse.bass_utils import run_bass_kernel_spmd

    input = np.asarray(input, dtype=f32)
    wts = _sep_weights(rois, offset)
    npix = np.array([len(t[1]) * len(t[2]) for t in wts])

    # octet sharding: sorted desc, octet r = 8 consecutive rois -> one per
    # core, so per-rank max (npad) is tight; balance per-core totals by
    # giving the octet's biggest roi to the least-loaded core
    order = np.argsort(-npix, kind="stable")
    npads = tuple(int(max(npix[order[8 * r]], 1)) for r in range(RPC))
    core_rois = [[None] * RPC for _ in range(N_CORES)]
    totals = np.zeros(N_CORES, np.int64)
    for r in range(RPC):
        members = order[8 * r:8 * r + 8]  # desc size
        dst = np.argsort(totals, kind="stable")  # asc load
        for j, m in enumerate(members):
            core_rois[dst[j]][r] = int(m)
            totals[dst[j]] += npix[m]

    key = npads
    if key not in _prog_cache:
        _prog_cache[key] = _build_program(npads)
    nc, plan = _prog_cache[key]
    T = len(plan["chunks"])
    PAIRS = plan["pairs"]

    fcl = np.ascontiguousarray(
        input.transpose(0, 2, 3, 1).astype(np.float16)
    ).reshape(B * H * W, C)

    in_maps = []
    for k in range(N_CORES):
        logical = np.zeros(T * 128, np.int32)
        a_arr = np.zeros((128, PAIRS, NJ), np.float16)
        for ci, segs in enumerate(plan["chunks"]):
            for (r, r0, r1, px0, pr) in segs:
                roi = core_rois[k][r]
                b, ys, xs, Wy, Wx = wts[roi]
                ny, nx = len(ys), len(xs)
                np_r = ny * nx
                n = r1 - r0
                pix = px0 + np.arange(n)
                m = pix < np_r
                if not m.any():
                    continue
                pixm = pix[m]
                yy = ys[pixm // nx]
                xx = xs[pixm % nx]
                logical[ci * 128 + r0:ci * 128 + r1][m] = (
                    b * (H * W) + yy.astype(np.int32) * W + xx
                )
                a_arr[np.arange(r0, r1)[m], pr, :] = (
                    Wy[pixm // nx] * Wx[pixm % nx]
                ).astype(np.float16)
        idx16 = np.tile(logical.astype(np.int16).reshape(-1, 16).T, (8, 1))
        in_maps.append({"fcl": fcl, "amat": a_arr, "pidx": idx16})

    res = run_bass_kernel_spmd(nc, in_maps, core_ids=list(range(N_CORES)))

    out_full = np.empty((N_ROIS, C, P, P), f32)
    for k in range(N_CORES):
        arr = res.results[k]["out"].astype(f32)  # (128, 2, RPC, 49)
        t = arr.transpose(2, 1, 0, 3).reshape(RPC, C, P, P)
        for r in range(RPC):
            out_full[core_rois[k][r]] = t[plan["rank2slot"][r]]
    return out_full


# revision 17
# speedup vs baseline: 1.0958x; 1.0146x over previous
"""DCNv2 deformable PS-RoI pooling on 8 Trainium2 NeuronCores (v2).

Strategy (RoI-data-parallel, 32 rois per core):
  * Host replicates the reference coordinate math exactly in float32.
    Bilinear weights / validity / 1-over-count factor per-bin separably:
    A[(y,x), j] = Wy[y, j] * Wx[x, j], so only pixels with
    (Wy row nonzero) x (Wx col nonzero) are needed -- the touched set is
    exactly a cartesian product ys x xs (~55% of the padded bbox).
  * Touched pixels of the 32 rois are bin-packed densely into 128-row
    chunks (per psum-group of rois), removing per-roi 128-padding.
  * Device (SPMD): indirect-DMA gather of pixel channel vectors
    (512B each, full DMA efficiency), matmul patch^T @ A per chunk into
    per-group PSUM banks, PSUM->SBUF copies split across DVE/Act,
    per-group output DMAs. All DMACopies dispatch from the Pool queue
    (cheap 25ns dispatch).
"""
import numpy as np

f32 = np.float32
f64 = np.float64

B, C, H, W = 8, 256, 64, 64
N_ROIS, P, S = 256, 7, 4
NJ = P * P  # 49
SCALE = f32(1.0 / 16.0)
TRANS_STD = f32(0.1)
N_CORES = 8
RPC = N_ROIS // N_CORES  # 32 rois per core
GROUP_SIZES = (6, 10, 10, 6)  # psum groups (<=10 per half-bank: 10*49<=512)
# processing order: small group first (PE starts early), big mixed middle
# (rems pack well), small group last (short drain tail)
GROUP_RANKS = (
    tuple(range(20, 26)),
    tuple(r for r in range(20) if r % 4 in (0, 3)),
    tuple(r for r in range(20) if r % 4 in (1, 2)),
    tuple(range(26, 32)),
)
N_GCALLS = 4  # one gather call per group

_prog_cache = {}


# --------------------------------------------------------------------------
# host math: exact f32 replication, separable per-bin weights
# --------------------------------------------------------------------------
def _sep_weights(rois, offset):
    """Per roi: (batch, ys, xs, Wy (ny,49) f64, Wx (nx,49) f64)."""
    rois = np.asarray(rois, dtype=f32)
    offset = np.asarray(offset, dtype=f32)
    N = rois.shape[0]
    batch = rois[:, 0].astype(np.int32)

    roi_sw = np.round(rois[:, 1]) * SCALE - f32(0.5)
    roi_sh = np.round(rois[:, 2]) * SCALE - f32(0.5)
    roi_ew = (np.round(rois[:, 3]) + f32(1.0)) * SCALE - f32(0.5)
    roi_eh = (np.round(rois[:, 4]) + f32(1.0)) * SCALE - f32(0.5)
    roi_w = np.maximum(roi_ew - roi_sw, f32(0.1))
    roi_h = np.maximum(roi_eh - roi_sh, f32(0.1))
    bin_w = roi_w / f32(P)
    bin_h = roi_h / f32(P)
    sub_w = bin_w / f32(S)
    sub_h = bin_h / f32(S)

    ph = np.arange(P, dtype=np.int32)
    pw = np.arange(P, dtype=np.int32)
    part_h = np.clip(
        np.floor(ph.astype(f32) / f32(P) * f32(P)).astype(np.int32), 0, P - 1
    )
    part_w = np.clip(
        np.floor(pw.astype(f32) / f32(P) * f32(P)).astype(np.int32), 0, P - 1
    )
    tx = offset[:, 0][:, part_h[:, None], part_w[None, :]] * TRANS_STD  # (N,7,7)
    ty = offset[:, 1][:, part_h[:, None], part_w[None, :]] * TRANS_STD

    wstart = (
        pw.astype(f32)[None, None, :] * bin_w[:, None, None]
        + roi_sw[:, None, None]
        + tx * roi_w[:, None, None]
    )  # (N,7,7)
    hstart = (
        ph.astype(f32)[None, :, None] * bin_h[:, None, None]
        + roi_sh[:, None, None]
        + ty * roi_h[:, None, None]
    )

    samp = np.arange(S, dtype=f32)
    wpos = wstart[..., None] + samp * sub_w[:, None, None, None]  # (N,7,7,4)
    hpos = hstart[..., None] + samp * sub_h[:, None, None, None]

    vw = (wpos >= f32(-0.5)) & (wpos <= f32(W) - f32(0.5))
    vh = (hpos >= f32(-0.5)) & (hpos <= f32(H) - f32(0.5))
    wc = np.clip(wpos, f32(0.0), f32(W - 1.0))
    hc = np.clip(hpos, f32(0.0), f32(H - 1.0))

    x0 = np.floor(wc).astype(np.int64)
    x1 = np.ceil(wc).astype(np.int64)
    y0 = np.floor(hc).astype(np.int64)
    y1 = np.ceil(hc).astype(np.int64)
    dx = (wc - np.floor(wc)).astype(f64)
    dy = (hc - np.floor(hc)).astype(f64)

    cnt_h = vh.sum(axis=3)  # (N,7,7)
    cnt_w = vw.sum(axis=3)
    ch = 1.0 / np.maximum(cnt_h, 1).astype(f64)
    cw = 1.0 / np.maximum(cnt_w, 1).astype(f64)

    jidx = (ph[:, None] * P + pw[None, :]).astype(np.int64)  # (7,7)
    jb = np.broadcast_to(jidx[None, :, :, None], (N, P, P, S))
    nb = np.broadcast_to(np.arange(N, dtype=np.int64)[:, None, None, None],
                         (N, P, P, S))

    Wy_full = np.zeros((N, H, NJ), f64)
    why0 = (1.0 - dy) * vh * ch[..., None]
    why1 = dy * vh * ch[..., None]
    np.add.at(Wy_full, (nb, y0, jb), why0)
    np.add.at(Wy_full, (nb, y1, jb), why1)

    Wx_full = np.zeros((N, W, NJ), f64)
    wwx0 = (1.0 - dx) * vw * cw[..., None]
    wwx1 = dx * vw * cw[..., None]
    np.add.at(Wx_full, (nb, x0, jb), wwx0)
    np.add.at(Wx_full, (nb, x1, jb), wwx1)

    out = []
    for n in range(N):
        ys = np.nonzero(np.abs(Wy_full[n]).sum(axis=1) > 0)[0]
        xs = np.nonzero(np.abs(Wx_full[n]).sum(axis=1) > 0)[0]
        out.append((int(batch[n]), ys.astype(np.int32), xs.astype(np.int32),
                    Wy_full[n][ys], Wx_full[n][xs]))
    return out


# --------------------------------------------------------------------------
# packing plan (shared structure across the 8 cores)
# --------------------------------------------------------------------------
def _make_plan(npads):
    """npads: tuple of 32 ints (desc), rank r -> padded pixel count.

    Returns dict with:
      rank2slot: rank -> output slot index o (0..31); slot o in group g
                 at position pos (o = base[g] + pos, pos = order of first
                 appearance in the group's chunk stream)
      chunks: list of [segments] in stream order; segment =
              (rank, row0, row1, pix0, pair)
      gcall_nch: chunks per gather call
      mchains: per group, list (PE-emission order) of
               (pos0, [(chunk, pair0, npair), ...]) accumulation chains
    """
    base = [0]
    for gs in GROUP_SIZES:
        base.append(base[-1] + gs)
    pair = 0
    all_chunks = []   # list of [ (rank, row0, row1, pix0, pair) ]
    group_chunk_rng = []
    rank2slot = {}
    chain_of_rank = {r: [] for r in range(len(npads))}
    for g, ranks in enumerate(GROUP_RANKS):
        c_start = len(all_chunks)
        ranks_sorted = sorted(ranks, key=lambda r: -npads[r])
        ded = []
        rems = []
        for r in ranks_sorted:
            nfull, rem = divmod(npads[r], 128)
            for i in range(nfull):
                ded.append([(r, 0, 128, i * 128)])
            rems.append((r, max(rem, 1) if nfull == 0 else rem, nfull))
        rems = [t for t in rems if t[1] > 0]
        rems.sort(key=lambda t: -t[1])
        bins = []
        for r, rem, nfull in rems:
            for b in bins:
                if b[0] + rem <= 128:
                    b[1].append((r, b[0], b[0] + rem, nfull * 128))
                    b[0] += rem
                    break
            else:
                bins.append([rem, [(r, 0, rem, nfull * 128)]])
        # bins first: single-chunk chains complete as soon as their
        # chunk lands, giving PE early work; dedicated chunks after
        gchunks = [b[1] for b in bins] + ded
        # assign pairs + positions (first appearance order) + chains
        pos_of = {}
        for segs in gchunks:
            ci = len(all_chunks)
            segs2 = []
            for (r, r0, r1, px) in segs:
                if r not in pos_of:
                    pos_of[r] = len(pos_of)
                segs2.append((r, r0, r1, px, pair))
                chain_of_rank[r].append((px, ci, pair))
                pair += 1
            all_chunks.append(segs2)
        for r, pos in pos_of.items():
            rank2slot[r] = base[g] + pos
        group_chunk_rng.append((c_start, len(all_chunks)))

    T = len(all_chunks)
    # one gather call per group
    gc_bounds = [0] + [rng[1] for rng in group_chunk_rng]
    gcall_nch = [gc_bounds[i + 1] - gc_bounds[i] for i in range(N_GCALLS)]

    # per-group chains in pos order, then merge runs of single-chunk
    # chains with consecutive (pos, pair) in the same chunk
    mchains = []
    for g, ranks in enumerate(GROUP_RANKS):
        glist = [None] * len(ranks)
        for r in ranks:
            pos = rank2slot[r] - base[g]
            # emission/accumulation order = chunk order (data availability)
            glist[pos] = sorted(chain_of_rank[r], key=lambda t: t[1])
        merged = []
        pos = 0
        while pos < len(glist):
            ch = glist[pos]
            if len(ch) == 1:
                px, ci, pr = ch[0]
                npair = 1
                while (pos + npair < len(glist)
                       and len(glist[pos + npair]) == 1
                       and glist[pos + npair][0][1] == ci
                       and glist[pos + npair][0][2] == pr + npair):
                    npair += 1
                merged.append((pos, [(ci, pr, npair)]))
                pos += npair
            else:
                merged.append((pos, [(ci, pr, 1) for (px, ci, pr) in ch]))
                pos += 1
        # PE emission order: chains whose data lands earliest first
        merged.sort(key=lambda m: max(it[0] for it in m[1]))
        mchains.append(merged)

    return dict(rank2slot=rank2slot, chunks=all_chunks,
                gcall_nch=gcall_nch, pairs=pair, mchains=mchains,
                base=base)


# --------------------------------------------------------------------------
# device program
# --------------------------------------------------------------------------
def _build_program(npads):
    import concourse.bacc as bacc
    import concourse.mybir as mybir
    from concourse.tile import TileContext

    plan = _make_plan(npads)
    T = len(plan["chunks"])
    PAIRS = plan["pairs"]
    gcall_nch = plan["gcall_nch"]
    base = plan["base"]

    nc = bacc.Bacc("TRN2", num_devices=N_CORES)
    dt = mybir.dt
    fcl = nc.dram_tensor("fcl", [B * H * W, C], dt.float16, kind="ExternalInput")
    amat = nc.dram_tensor("amat", [128, PAIRS, NJ], dt.float16,
                          kind="ExternalInput")
    ca = gcall_nch[0]
    pidxa = nc.dram_tensor("pidxa", [128, ca * 8], dt.int16,
                           kind="ExternalInput")
    pidxb = nc.dram_tensor("pidxb", [128, (T - ca) * 8], dt.int16,
                           kind="ExternalInput")
    outd = nc.dram_tensor("out", [128, 2, RPC, NJ], dt.float16,
                          kind="ExternalOutput")

    gc_bounds = [0]
    for n in gcall_nch:
        gc_bounds.append(gc_bounds[-1] + n)

    def gc_of(ci):
        for gc in range(N_GCALLS):
            if ci < gc_bounds[gc + 1]:
                return gc
        raise AssertionError(ci)

    with TileContext(nc) as tc:
        with (
            tc.tile_pool(name="main", bufs=1) as mp,
            tc.tile_pool(name="psum", bufs=1, space="PSUM") as pp,
        ):
            idxa_t = mp.tile([128, ca * 8], dt.int16, tag="idxa")
            idxb_t = mp.tile([128, (T - ca) * 8], dt.int16, tag="idxb")
            a_t = mp.tile([128, PAIRS, NJ], dt.float16, tag="amat")
            ob = mp.tile([128, 2, RPC, NJ], dt.float16, tag="outbuf")
            warm = mp.tile([128, 8], dt.float16, tag="warm")
            nc.sync.dma_start(out=idxa_t[:], in_=pidxa[:])
            nc.sync.dma_start(out=idxb_t[:], in_=pidxb[:])
            nc.sync.dma_start(out=a_t[:], in_=amat[:])
            # preload the Act engine's activation table (1283ns) upfront
            nc.vector.memset(warm[:], 0.0)
            nc.scalar.activation(warm[:], warm[:],
                                 mybir.ActivationFunctionType.Copy)

            patch = []
            for gc in range(N_GCALLS):
                nch = gcall_nch[gc]
                p_t = mp.tile([128, nch, C], dt.float16, tag=f"patch{gc}")
                c0 = gc_bounds[gc]
                idxs = (idxa_t[:, :] if gc == 0
                        else idxb_t[:, (c0 - ca) * 8:(c0 - ca + nch) * 8])
                nc.gpsimd.dma_gather(
                    out_ap=p_t[:],
                    in_ap=fcl[:],
                    idxs_ap=idxs,
                    num_idxs=nch * 128,
                    num_idxs_reg=nch * 128,
                    elem_size=C,
                    single_packet=False,
                )
                patch.append(p_t)

            pb = {}
            for g, gs in enumerate(GROUP_SIZES):
                for h in range(2):
                    pb[(g, h)] = pp.tile([128, gs, NJ], dt.float32,
                                         tag=f"pb{g}_{h}",
                                         name=f"pb{g}_{h}")

            for g in range(len(GROUP_SIZES)):
                for (pos0, items) in plan["mchains"][g]:
                    for h in range(2):
                        for i, (ci, pr, npair) in enumerate(items):
                            gc = gc_of(ci)
                            lc = ci - gc_bounds[gc]
                            nc.tensor.matmul(
                                out=pb[(g, h)][:, pos0:pos0 + npair, :],
                                lhsT=patch[gc][:, lc, h * 128:(h + 1) * 128],
                                rhs=a_t[:, pr:pr + npair, :],
                                start=(i == 0),
                                stop=(i == len(items) - 1),
                            )

            for g, gs in enumerate(GROUP_SIZES):
                o0 = base[g]
                nc.vector.tensor_copy(out=ob[:, 0, o0:o0 + gs, :],
                                      in_=pb[(g, 0)][:])
                nc.scalar.activation(ob[:, 1, o0:o0 + gs, :], pb[(g, 1)][:],
                                     mybir.ActivationFunctionType.Copy)
                nc.sync.dma_start(out=outd[:, :, o0:o0 + gs, :],
                                  in_=ob[:, :, o0:o0 + gs, :])
    nc.compile()
    return nc, plan


# --------------------------------------------------------------------------
# entry point
# --------------------------------------------------------------------------
def kernel(input, rois, offset):
    from concourse.bass_utils import run_bass_kernel_spmd

    input = np.asarray(input, dtype=f32)
    wts = _sep_weights(rois, offset)
    npix = np.array([len(t[1]) * len(t[2]) for t in wts])

    # octet sharding: sorted desc, octet r = 8 consecutive rois -> one per
    # core, so per-rank max (npad) is tight; balance per-core totals by
    # giving the octet's biggest roi to the least-loaded core
    order = np.argsort(-npix, kind="stable")
    npads = tuple(int(max(npix[order[8 * r]], 1)) for r in range(RPC))
    core_rois = [[None] * RPC for _ in range(N_CORES)]
    totals = np.zeros(N_CORES, np.int64)
    for r in range(RPC):
        members = order[8 * r:8 * r + 8]  # desc size
        dst = np.argsort(totals, kind="stable")  # asc load
        for j, m in enumerate(members):
            core_rois[dst[j]][r] = int(m)
            totals[dst[j]] += npix[m]

    key = npads
    if key not in _prog_cache:
        _prog_cache[key] = _build_program(npads)
    nc, plan = _prog_cache[key]
    T = len(plan["chunks"])
    PAIRS = plan["pairs"]

    fcl = np.ascontiguousarray(
        input.transpose(0, 2, 3, 1).astype(np.float16)
    ).reshape(B * H * W, C)

    in_maps = []
    for k in range(N_CORES):
        logical = np.zeros(T * 128, np.int32)
        a_arr = np.zeros((128, PAIRS, NJ), np.float16)
        for ci, segs in enumerate(plan["chunks"]):
            for (r, r0, r1, px0, pr) in segs:
                roi = core_rois[k][r]
                b, ys, xs, Wy, Wx = wts[roi]
                ny, nx = len(ys), len(xs)
                np_r = ny * nx
                n = r1 - r0
                pix = px0 + np.arange(n)
                m = pix < np_r
                if not m.any():
                    continue
                pixm = pix[m]
                yy = ys[pixm // nx]
                xx = xs[pixm % nx]
                logical[ci * 128 + r0:ci * 128 + r1][m] = (
                    b * (H * W) + yy.astype(np.int32) * W + xx
                )
                a_arr[np.arange(r0, r1)[m], pr, :] = (
                    Wy[pixm // nx] * Wx[pixm % nx]
                ).astype(np.float16)
        idx16 = np.tile(logical.astype(np.int16).reshape(-1, 16).T, (8, 1))
        ca = plan["gcall_nch"][0]
        in_maps.append({"fcl": fcl, "amat": a_arr,
                        "pidxa": np.ascontiguousarray(idx16[:, :ca * 8]),
                        "pidxb": np.ascontiguousarray(idx16[:, ca * 8:])})

    res = run_bass_kernel_spmd(nc, in_maps, core_ids=list(range(N_CORES)))

    out_full = np.empty((N_ROIS, C, P, P), f32)
    for k in range(N_CORES):
        arr = res.results[k]["out"].astype(f32)  # (128, 2, RPC, 49)
        t = arr.transpose(2, 1, 0, 3).reshape(RPC, C, P, P)
        for r in range(RPC):
            out_full[core_rois[k][r]] = t[plan["rank2slot"][r]]
    return out_full
